# revision 14
# baseline (speedup 1.0000x reference)
"""KiloNeRF Trainium2 kernel: 4096 tiny MLPs, 512 points each, 8 NeuronCores.

Sharding: expert-parallel along the network axis (512 nets/core). Host-side
numpy packs per-core inputs into feature-major, PE-friendly layouts; the
device kernel is a stream of full-array block-diagonal matmuls (4 nets per
128-partition tile), bf16 inputs with f32 PSUM accumulation.

v5:
- Feature layer folded into direction layer (7 matmuls / group).
- Transfers batched per supergroup of 8 groups, double-buffered, emitted one
  supergroup ahead of compute; issue split across both HWDGE rings.
- Dense weight canvases (no 64B-run descriptor storms, no memset).
- Compact Lout (16 out cols, 4 groups per PSUM bank); its eviction runs on
  the otherwise-idle GpSimd engine.
- L0 emitted one group ahead (software pipelining); h3 eviction split
  between Vector and Scalar to halve its latency on the PE critical path.
"""

import sys

sys.path.insert(0, "/opt/trn_rl_repo")

import numpy as np
import ml_dtypes

N_NET = 4096
P = 512
PC = 63
DC = 27
H = 32
NCORES = 8
NPC = N_NET // NCORES  # nets per core = 512
NPG = 4  # nets per group (one 128-partition tile)
G = NPC // NPG  # groups per core = 128
R = 8  # groups per supergroup
SG = G // R  # supergroups = 16

BF16 = ml_dtypes.bfloat16

_nc_cache = {}


def _build_nc():
    import concourse.mybir as mybir
    import concourse.tile as tile
    from concourse import bacc

    nc = bacc.Bacc("TRN2")
    dt = mybir.dt
    AF = mybir.ActivationFunctionType
    ALU = mybir.AluOpType

    XW = 3 * P  # x cols per group: pos0 | pos1 | dir
    CW = 3 * 128  # dense canvas cols per group: w1 | wfd | wdd
    LW = 32  # compact lout canvas cols per group: wr(16) | wa(16)
    HP = P // 2

    with tile.TileContext(nc) as tc:
        x_d = nc.dram_tensor("xin", [SG, 128, R * XW], dt.bfloat16, kind="ExternalInput")
        w0_d = nc.dram_tensor("w0", [SG, 128, R * 128], dt.bfloat16, kind="ExternalInput")
        cv_d = nc.dram_tensor("cv", [SG, 128, R * CW], dt.bfloat16, kind="ExternalInput")
        lo_d = nc.dram_tensor("lo", [SG, 128, R * LW], dt.bfloat16, kind="ExternalInput")
        biasg_d = nc.dram_tensor("biasg", [128, G * 4], dt.float32, kind="ExternalInput")
        boutb_d = nc.dram_tensor("boutb", [128, G // 4], dt.float32, kind="ExternalInput")
        out_d = nc.dram_tensor("out", [SG, 4, 16, 2, P], dt.float32, kind="ExternalOutput")

        with (
            tc.tile_pool(name="big", bufs=1) as bigp,
            tc.tile_pool(name="act", bufs=5) as actp,
            tc.tile_pool(name="ob", bufs=3) as obp,
            tc.tile_pool(name="ps0", bufs=2, space="PSUM") as ps0,
            tc.tile_pool(name="ps1", bufs=2, space="PSUM") as ps1,
            tc.tile_pool(name="psd", bufs=2, space="PSUM") as psd,
            tc.tile_pool(name="pso", bufs=2, space="PSUM") as pso,
        ):
            biasg = bigp.tile([128, G * 4], dt.float32, tag="biasg")
            boutb = bigp.tile([128, G // 4], dt.float32, tag="boutb")
            nc.sync.dma_start(out=biasg[:], in_=biasg_d[:])
            nc.scalar.dma_start(out=boutb[:], in_=boutb_d[:])

            xbig = [bigp.tile([128, R * XW], dt.bfloat16, tag=f"x{i}", name=f"x{i}") for i in range(2)]
            w0big = [bigp.tile([128, R * 128], dt.bfloat16, tag=f"w0{i}", name=f"w0{i}") for i in range(2)]
            cvbig = [bigp.tile([128, R * CW], dt.bfloat16, tag=f"cv{i}", name=f"cv{i}") for i in range(2)]
            lobig = [bigp.tile([128, R * LW], dt.bfloat16, tag=f"lo{i}", name=f"lo{i}") for i in range(2)]

            def emit_dma(s):
                b = s % 2
                xb, w0b, cvb, lob = xbig[b], w0big[b], cvbig[b], lobig[b]
                half = R * XW // 2
                if s == 0:
                    # group-0 pieces first so compute starts ~immediately,
                    # then the rest in big chunks (small weights before big x
                    # on each FIFO ring)
                    nc.scalar.dma_start(out=w0b[:, 0:128], in_=w0_d[s, :, 0:128])
                    nc.scalar.dma_start(out=cvb[:, 0:CW], in_=cv_d[s, :, 0:CW])
                    nc.scalar.dma_start(out=lob[:, 0:LW], in_=lo_d[s, :, 0:LW])
                    nc.sync.dma_start(out=xb[:, 0:XW], in_=x_d[s, :, 0:XW])
                    nc.scalar.dma_start(out=w0b[:, 128:], in_=w0_d[s, :, 128:])
                    nc.scalar.dma_start(out=cvb[:, CW:], in_=cv_d[s, :, CW:])
                    nc.scalar.dma_start(out=lob[:, LW:], in_=lo_d[s, :, LW:])
                    nc.sync.dma_start(out=xb[:, XW:half], in_=x_d[s, :, XW:half])
                    nc.scalar.dma_start(out=xb[:, half:], in_=x_d[s, :, half:])
                    return
                # small weights first on each ring, then the big x halves
                nc.sync.dma_start(out=w0b[:], in_=w0_d[s])
                nc.scalar.dma_start(out=cvb[:], in_=cv_d[s])
                nc.scalar.dma_start(out=lob[:], in_=lo_d[s])
                nc.sync.dma_start(out=xb[:, 0:half], in_=x_d[s, :, 0:half])
                nc.scalar.dma_start(out=xb[:, half:], in_=x_d[s, :, half:])

            state = {}  # per-bank psum tile + obstage

            def emit_group(s, r):
                b = s % 2
                g = R * s + r
                h, q = r // 4, r % 4
                xb, w0b, cvb, lob = xbig[b], w0big[b], cvbig[b], lobig[b]
                pos0 = xb[:, r * XW : r * XW + P]
                pos1 = xb[:, r * XW + P : r * XW + 2 * P]
                dirt = xb[:, r * XW + 2 * P : r * XW + 3 * P]
                w0 = w0b[:, r * 128 : (r + 1) * 128]
                cv = cvb[:, r * CW : (r + 1) * CW]

                # --- L0: h1 = relu(pos @ W0^T + b0) ---
                p_l0 = ps0.tile([128, P], dt.float32, tag="l0")
                nc.tensor.matmul(p_l0[0:64], lhsT=w0[:, 0:64], rhs=pos0, start=True, stop=True)
                nc.tensor.matmul(p_l0[64:128], lhsT=w0[:, 64:128], rhs=pos1, start=True, stop=True)
                h1 = actp.tile([128, P], dt.bfloat16, tag="h1")
                nc.scalar.activation(
                    h1[:], p_l0[:], AF.Relu, bias=biasg[:, 4 * g : 4 * g + 1], scale=1.0
                )

                def bia(i):
                    return biasg[:, 4 * g + i : 4 * g + i + 1]

                # --- L1: h2 = relu(h1 @ W1^T + b1) ---
                p_l1 = ps1.tile([128, P], dt.float32, tag="l1")
                nc.tensor.matmul(p_l1[:], lhsT=cv[:, 0:128], rhs=h1[:], start=True, stop=True)
                h2 = actp.tile([128, P], dt.bfloat16, tag="h2")
                nc.vector.tensor_scalar(h2[:], p_l1[:], bia(1), 0.0, op0=ALU.add, op1=ALU.max)

                # --- Ld: h3 = relu(h2 @ Wfd^T + dir @ Wdd^T + bfd) ---
                p_ld = psd.tile([128, P], dt.float32, tag="ld")
                nc.tensor.matmul(p_ld[:], lhsT=cv[:, 128:256], rhs=h2[:], start=True, stop=False)
                nc.tensor.matmul(p_ld[:], lhsT=cv[:, 256:384], rhs=dirt, start=False, stop=True)
                h3 = actp.tile([128, P], dt.bfloat16, tag="h3")
                # split halves across both engines to halve the latency
                nc.vector.tensor_scalar(
                    h3[:, 0:HP], p_ld[:, 0:HP], bia(2), 0.0, op0=ALU.add, op1=ALU.max
                )
                nc.scalar.activation(
                    h3[:, HP:P], p_ld[:, HP:P], AF.Relu, bias=bia(2), scale=1.0
                )

                # --- Lout (compact, 4 groups per PSUM bank):
                #     psum rows 32q+4j+k = net j, chan k (rgb,alpha) ---
                if q == 0:
                    state["lo4", h] = pso.tile([128, P], dt.float32, tag="lo4", name="p_lo4")
                p_lo4 = state["lo4", h]
                nc.tensor.matmul(
                    p_lo4[32 * q : 32 * q + 16],
                    lhsT=lob[:, r * LW : r * LW + 16],
                    rhs=h3[:],
                    start=True,
                    stop=False,
                    tile_position=(0, 32 * q),
                )
                nc.tensor.matmul(
                    p_lo4[32 * q : 32 * q + 16],
                    lhsT=lob[:, r * LW + 16 : r * LW + 32],
                    rhs=h2[:],
                    start=False,
                    stop=True,
                    tile_position=(0, 32 * q),
                )
                if q == 3:
                    # one eviction per 4 groups; garbage rows are never read
                    obstage = state["obstage"]
                    nc.scalar.activation(
                        obstage[:, h * P : (h + 1) * P],
                        state.pop(("lo4", h))[:],
                        AF.Identity,
                        bias=boutb[:, 2 * s + h : 2 * s + h + 1],
                        scale=1.0,
                    )

            emit_dma(0)
            emit_dma(1)
            for s in range(SG):
                obstage = obp.tile([128, 2 * P], dt.float32, tag="obstage")
                state["obstage"] = obstage
                for r in range(R):
                    emit_group(s, r)
                # out-DMAs BEFORE the next prefetch so they aren't stuck
                # behind megabytes of x on the ring FIFOs
                for q in range(4):
                    eng = nc.sync if q % 2 == 0 else nc.scalar
                    eng.dma_start(
                        out=out_d[s, q],
                        in_=obstage[32 * q : 32 * q + 16].rearrange("p (h c) -> p h c", h=2),
                    )
                if s + 2 < SG:
                    emit_dma(s + 2)

    nc.compile()
    return nc


def _pack_core(c, x, W0, b0, W1, b1, Wa, ba, Wf, bf, Wd, bd, Wr, br):
    lo, hi = c * NPC, (c + 1) * NPC
    XW = 3 * P
    xT = np.ascontiguousarray(
        x[lo:hi].transpose(0, 2, 1)
    )  # [512, 90, 512] f32 feature-major

    # x big tiles: [SG, 128, R*(pos0|pos1|dir)]
    xbig = np.zeros((G, 128, 3, P), dtype=BF16)
    pt = xT[:, :PC, :].astype(BF16).reshape(G, 4, PC, P)
    xbig[:, 0:PC, 0] = pt[:, 0]
    xbig[:, 64 : 64 + PC, 0] = pt[:, 1]
    xbig[:, 0:PC, 1] = pt[:, 2]
    xbig[:, 64 : 64 + PC, 1] = pt[:, 3]
    dd = xT[:, PC:, :].astype(BF16).reshape(G, 4, DC, P)
    for j in range(4):
        xbig[:, 32 * j : 32 * j + DC, 2] = dd[:, j]
    xbig = np.ascontiguousarray(
        xbig.reshape(SG, R, 128, XW).transpose(0, 2, 1, 3).reshape(SG, 128, R * XW)
    )

    # L0 weights, feature-major lhsT canvas ([in,out] = W^T), 2 nets/canvas-half
    w0T = W0[lo:hi].transpose(0, 2, 1).astype(BF16).reshape(G, 4, PC, H)
    w0p = np.zeros((G, 128, 128), dtype=BF16)
    for j in range(4):
        r = 64 * (j % 2)
        w0p[:, r : r + PC, 32 * j : 32 * j + 32] = w0T[:, j]
    w0p = np.ascontiguousarray(
        w0p.reshape(SG, R, 128, 128).transpose(0, 2, 1, 3).reshape(SG, 128, R * 128)
    )

    # fold the (linear, non-output) feature layer into the direction layer:
    # Wfd = Wd_f @ Wf, bfd = Wd_f @ bf + bd
    Wd_f = Wd[lo:hi, :, :H]  # [n, 32(out), 32(feat-in)]
    Wfd = np.matmul(Wd_f, Wf[lo:hi])  # [n, 32(out), 32(h2-in)]
    bfd = np.einsum("nof,nf->no", Wd_f, bf[lo:hi]) + bd[lo:hi]

    # dense block-diagonal canvases [G, 128, 3 mats, 128]
    w1T = W1[lo:hi].transpose(0, 2, 1).astype(BF16).reshape(G, 4, H, H)
    wfdT = Wfd.transpose(0, 2, 1).astype(BF16).reshape(G, 4, H, H)
    wddT = Wd[lo:hi, :, H:].transpose(0, 2, 1).astype(BF16).reshape(G, 4, DC, H)
    cvd = np.zeros((G, 128, 3, 128), dtype=BF16)
    for j in range(4):
        sl = slice(32 * j, 32 * j + 32)
        cvd[:, sl, 0, sl] = w1T[:, j]
        cvd[:, sl, 1, sl] = wfdT[:, j]
        cvd[:, 32 * j : 32 * j + DC, 2, sl] = wddT[:, j]
    cvd = np.ascontiguousarray(
        cvd.reshape(SG, R, 128, 384).transpose(0, 2, 1, 3).reshape(SG, 128, R * 384)
    )

    # compact lout mini-canvases [G, 128, 2, 16]: wr cols 4j..4j+2, wa col 4j+3
    wrT = Wr[lo:hi].transpose(0, 2, 1).astype(BF16).reshape(G, 4, H, 3)
    waT = Wa[lo:hi].transpose(0, 2, 1).astype(BF16).reshape(G, 4, H, 1)
    lod = np.zeros((G, 128, 2, 16), dtype=BF16)
    for j in range(4):
        lod[:, 32 * j : 32 * j + 32, 0, 4 * j : 4 * j + 3] = wrT[:, j]
        lod[:, 32 * j : 32 * j + 32, 1, 4 * j + 3] = waT[:, j, :, 0]
    lod = np.ascontiguousarray(
        lod.reshape(SG, R, 128, 32).transpose(0, 2, 1, 3).reshape(SG, 128, R * 32)
    )

    # biases: per-group cols (b0, b1, bfd, -) and per-bank lout bias
    biasg = np.zeros((G, 128, 4), dtype=np.float32)
    biasg[:, :, 0] = b0[lo:hi].reshape(G, 128)
    biasg[:, :, 1] = b1[lo:hi].reshape(G, 128)
    biasg[:, :, 2] = bfd.reshape(G, 128)
    biasg = np.ascontiguousarray(biasg.transpose(1, 0, 2).reshape(128, G * 4))

    bo = np.zeros((G, 4, 4), dtype=np.float32)  # [group, net j, chan k]
    bo[:, :, 0:3] = br[lo:hi].reshape(G, 4, 3)
    bo[:, :, 3] = ba[lo:hi].reshape(G, 4)
    boutb = np.zeros((128, G // 4), dtype=np.float32)
    for q in range(4):
        # bank B holds groups 4B..4B+3; group 4B+q sits at rows 32q+4j+k
        boutb[32 * q : 32 * q + 16, :] = bo[q::4].reshape(G // 4, 16).T

    return {
        "xin": xbig,
        "w0": w0p,
        "cv": cvd,
        "lo": lod,
        "biasg": biasg,
        "boutb": boutb,
    }


def kernel(**inputs):
    from concourse.bass_utils import run_bass_kernel_spmd

    if "nc" not in _nc_cache:
        _nc_cache["nc"] = _build_nc()
    nc = _nc_cache["nc"]

    from concurrent.futures import ThreadPoolExecutor

    with ThreadPoolExecutor(max_workers=8) as ex:
        in_maps = list(ex.map(lambda c: _pack_core(c, **inputs), range(NCORES)))

    res = run_bass_kernel_spmd(nc, in_maps, core_ids=list(range(NCORES)))

    out = np.empty((N_NET, P, 4), dtype=np.float32)
    for c in range(NCORES):
        o = res.results[c]["out"]  # [SG, q, 16(4j+k), 2(h), P]
        o = o.reshape(SG, 4, 4, 4, 2, P)  # [s, q, j, k, h, c]
        o = o.transpose(0, 4, 1, 2, 5, 3)  # [s, h, q, j, c, k]
        out[c * NPC : (c + 1) * NPC] = o.reshape(NPC, P, 4)
    return out
